# revision 5
# baseline (speedup 1.0000x reference)
"""Symmetric-pair Trainium2 Bass kernel for nn_AdjacencyGenerator.

Same semantics as kernel.py, but each core computes only block-columns
delta = 0..4 of its row panel (80 PSUM tiles instead of 128):
  - delta 1,2,3: computed once per unordered pair; the transposed tile
    feeds "mirror" candidate extraction for the peer's rows, exchanged
    via one AllToAll with partition-id dynamic-offset DMAs.
  - delta 4 (self-paired distance) is computed unsymmetrized on both
    cores (an SPMD-uniform half-split of that block is impossible).
  - delta 0 (diagonal block) is local.
Candidate slot layout (32 subchunks x 8 slots per row) is identical to
kernel.py, so merge/scatter/output stages are unchanged.
"""
import sys

for _p in ("/opt/trn_rl_repo", "/root/.axon_site/_ro/trn_rl_repo"):
    if _p not in sys.path:
        sys.path.insert(0, _p)

import numpy as np

from concourse import bacc, mybir, bass
from concourse.tile import TileContext
from concourse.bass_utils import run_bass_kernel_spmd

F32 = mybir.dt.float32
BF16 = mybir.dt.bfloat16
F16 = mybir.dt.float16
U16 = mybir.dt.uint16
I16 = mybir.dt.int16
ALU = mybir.AluOpType
ACTF = mybir.ActivationFunctionType
AXL = mybir.AxisListType

N_CORES = 8
K1 = 21
MIRROR_DELTAS = (1, 2, 3)


def build_kernel(B=8192, D=1024):
    R = B // N_CORES            # rows per core (1024)
    MT = R // 128               # m-tiles per core (8)
    KT = D // 128               # contraction k-tiles (8)
    SLOTS = (B // 512) * 8      # candidate slots per row (128)
    OUT_CH = B // 1024          # output scatter chunks (8)
    NBLK = N_CORES // 2 + 1     # block-columns computed locally (5)

    nc = bacc.Bacc("TRN2", target_bir_lowering=False, debug=False,
                   num_devices=N_CORES)

    xall_ext = nc.declare_dram_parameter("xall", [B, D], F32, isOutput=False)
    ew_ext = nc.declare_dram_parameter("ew", [1, K1], F32, isOutput=False)
    out_ext = nc.declare_dram_parameter("out", [R, B], F32, isOutput=True)

    # AllToAll exchange buffers: per dest core 128 partitions x 384 f32
    # (256 f32 candidate values + 256 u16 indices packed as 128 f32)
    a2a_in = {g: nc.dram_tensor(f"a2a_in{g}", [N_CORES * 128, 192], F32)
              for g in "AB"}
    a2a_out = {g: nc.dram_tensor(f"a2a_out{g}", [N_CORES * 128, 192], F32)
               for g in "AB"}

    with TileContext(nc) as tc:
        with (
            tc.tile_pool(name="xl", bufs=1) as p_xl,
            tc.tile_pool(name="xr", bufs=1) as p_xr,
            tc.tile_pool(name="mt", bufs=1) as p_mt,
            tc.tile_pool(name="cand", bufs=1) as p_cand,
            tc.tile_pool(name="send", bufs=1) as p_send,
            tc.tile_pool(name="corrp", bufs=3) as p_corr,
            tc.tile_pool(name="stage", bufs=2) as p_stage,
            tc.tile_pool(name="quad", bufs=4) as p_quad,
            tc.tile_pool(name="small", bufs=2) as p_small,
            tc.tile_pool(name="const", bufs=1) as p_const,
            tc.tile_pool(name="pmm", bufs=4, space="PSUM") as p_mm,
            tc.tile_pool(name="ptr", bufs=2, space="PSUM") as p_tr,
            tc.tile_pool(name="pbc", bufs=1, space="PSUM") as p_bc,
        ):
            # ---------------- constants ----------------
            ident = p_const.tile([128, 128], F32, tag="ident")
            nc.vector.memset(ident[:, :], 1.0)
            nc.gpsimd.affine_select(ident[:, :], ident[:, :],
                                    pattern=[[-1, 128]],
                                    compare_op=ALU.is_equal, fill=0.0,
                                    base=0, channel_multiplier=1)
            base_iota = p_const.tile([128, SLOTS], U16, tag="base_iota")
            nc.gpsimd.iota(base_iota[:, :],
                           pattern=[[1024, 8], [512, 2], [0, 8]],
                           base=0, channel_multiplier=0)
            rank_iota = p_const.tile([128, 24], I16, tag="rank_iota")
            nc.gpsimd.iota(rank_iota[:, :], pattern=[[1, 24]], base=1,
                           channel_multiplier=0)

            # ------------- edge weights -> fp16 [128, 24] -------------
            ew_sb = p_const.tile([1, 64], F32, tag="ew_sb")
            nc.gpsimd.dma_start(out=ew_sb[:, 0:K1], in_=ew_ext[:, :])
            nc.vector.reduce_sum(ew_sb[:, 32:33], ew_sb[:, 0:K1], axis=AXL.X)
            nc.vector.tensor_scalar_add(ew_sb[:, 33:34], ew_sb[:, 32:33], 1e-8)
            nc.vector.reciprocal(ew_sb[:, 34:35], ew_sb[:, 33:34])
            ew_n = p_const.tile([1, 24], F32, tag="ew_n")
            nc.vector.memset(ew_n[:, :], 0.0)
            nc.vector.tensor_scalar_mul(ew_n[:, 0:K1], ew_sb[:, 0:K1],
                                        ew_sb[:, 34:35])
            ones1 = p_const.tile([1, 128], F32, tag="ones1")
            nc.vector.memset(ones1[:, :], 1.0)
            ew_ps = p_bc.tile([128, 24], F32, tag="ew_ps")
            nc.tensor.matmul(ew_ps[:, :], ones1[:, :], ew_n[:, :],
                             start=True, stop=True)
            ew16 = p_const.tile([128, 24], F16, tag="ew16")
            nc.scalar.copy(out=ew16[:, :], in_=ew_ps[:, :])

            # ---------------- persistent tiles ----------------
            # xl: own rows transposed (lhsT + delta-0 rhs), 2 planes x 2
            # chunks of 512 rows.  xr: rotating rhs buffers, 4 slots.
            xl = {}
            for pl in range(2):
                for q in range(2):
                    xl[(pl, q)] = p_xl.tile([128, KT * 512], BF16,
                                            name=f"xl{pl}_{q}", tag=f"xl{pl}_{q}")
            xr = {}
            for pl in range(2):
                for s in range(4):
                    xr[(pl, s)] = p_xr.tile([128, KT * 512], BF16,
                                            name=f"xr{pl}_{s}", tag=f"xr{pl}_{s}")
            # mirror assembly tiles: per q-chunk, [128 j x (4 g-groups x
            # 1024 i)] f32
            mt_q = {q: p_mt.tile([128, 4096], F32, name=f"mtq{q}", tag=f"mtq{q}")
                    for q in range(2)}
            cand_v = p_cand.tile([128, MT * SLOTS], F32, tag="cand_v")
            cand_w = p_cand.tile([128, MT * SLOTS], U16, tag="cand_w")
            # recv staging (separate tiles: no write-aliasing with local
            # extraction writes to cand_v/cand_w)
            rv = p_cand.tile([128, MT * 48], F32, tag="rv")
            rw = p_cand.tile([128, MT * 48], U16, tag="rw")
            # send staging per mirror delta
            sv = {d: p_send.tile([128, 128], F32, name=f"sv{d}", tag=f"sv{d}")
                  for d in MIRROR_DELTAS}
            si = {d: p_send.tile([128, 128], U16, name=f"si{d}", tag=f"si{d}")
                  for d in MIRROR_DELTAS}

            # ---------------- prep: 512 rows -> hi/lo planes ----------
            def prep_rows(row0, dhi, dlo, first=False, use_dve=False):
                xns = []
                for qq in range(4):
                    r0 = row0 + qq * 128
                    xst = p_stage.tile([128, D], F32, tag="xst")
                    nc.sync.dma_start(out=xst[:, :],
                                      in_=xall_ext[r0:r0 + 128, :])
                    ssc = p_small.tile([128, 4], F32, tag="ssc")
                    sq = p_stage.tile([128, D], BF16, tag="sq", bufs=1)
                    nc.scalar.activation(sq[:, :], xst[:, :], ACTF.Square,
                                         accum_out=ssc[:, 0:1])
                    nc.scalar.activation(ssc[:, 1:2], ssc[:, 0:1], ACTF.Sqrt)
                    nc.vector.reciprocal(ssc[:, 2:3], ssc[:, 1:2])
                    xn = p_quad.tile([128, D], F32, tag="xn")
                    if use_dve or (first and qq == 0):
                        nc.vector.tensor_scalar_mul(xn[:, :], xst[:, :],
                                                    ssc[:, 2:3])
                    else:
                        nc.scalar.activation(xn[:, :], xst[:, :], ACTF.Copy,
                                             scale=ssc[:, 2:3])
                    xns.append(xn)
                for kt in range(KT):
                    ps = p_tr.tile([128, 512], F32, tag="ps_tr")
                    for qq in range(4):
                        nc.tensor.transpose(ps[:, qq * 128:(qq + 1) * 128],
                                            xns[qq][:, kt * 128:(kt + 1) * 128],
                                            ident[:, :])
                    hi_sl = dhi[:, kt * 512:(kt + 1) * 512]
                    nc.scalar.copy(out=hi_sl, in_=ps[:, :])
                    nc.vector.tensor_tensor(
                        out=dlo[:, kt * 512:(kt + 1) * 512],
                        in0=ps[:, :], in1=hi_sl, op=ALU.subtract)

            def prep_block(blk, first=False):
                # rows [R*blk, R*(blk+1)) -> dest buffers
                for q in range(2):
                    if blk == 0:
                        dhi, dlo = xl[(0, q)], xl[(1, q)]
                    else:
                        s = SLOT0[blk] + q
                        dhi, dlo = xr[(0, s)], xr[(1, s)]
                    prep_rows(R * blk + 512 * q, dhi, dlo,
                              first=(first and q == 0), use_dve=(blk <= 1))

            SLOT0 = {1: 0, 2: 2, 3: 0, 4: 2}

            def rhs_buf(blk, q):
                if blk == 0:
                    return {pl: xl[(pl, q)] for pl in range(2)}
                return {pl: xr[(pl, SLOT0[blk] + q)] for pl in range(2)}

            def lhs_slice(pl, kt, m):
                return xl[(pl, m // 4)][:, kt * 512 + (m % 4) * 128:
                                        kt * 512 + (m % 4) * 128 + 128]

            # ---------------- adj tile + extraction ----------------
            def do_tile(blk, q, m, mirror, ret_pmm=False):
                rb = rhs_buf(blk, q)
                pmm = p_mm.tile([128, 512], F32, tag="pmm")
                prods = [(0, 0), (0, 1), (1, 0)]
                order = [(kt, pa, pb) for kt in range(KT)
                         for pa, pb in prods[:2]]
                order += [(kt, 1, 0) for kt in range(KT)]
                for i, (kt, pa, pb) in enumerate(order):
                    nc.tensor.matmul(pmm[:, :], lhs_slice(pa, kt, m),
                                     rb[pb][:, kt * 512:(kt + 1) * 512],
                                     start=(i == 0),
                                     stop=(i == len(order) - 1))
                slot0 = m * SLOTS + blk * 16 + q * 8
                nc.vector.max(cand_v[:, slot0:slot0 + 8], pmm[:, :])
                nc.vector.max_index(cand_w[:, slot0:slot0 + 8],
                                    cand_v[:, slot0:slot0 + 8], pmm[:, :])
                if mirror:
                    ev = p_stage.tile([128, 512], F32, tag="ev", bufs=2)
                    nc.scalar.copy(out=ev[:, :], in_=pmm[:, :])
                    trp = p_tr.tile([128, 512], F32, tag="trp", bufs=1)
                    for g in range(4):
                        nc.tensor.transpose(trp[:, g * 128:(g + 1) * 128],
                                            ev[:, g * 128:(g + 1) * 128],
                                            ident[:, :])
                    # piece g -> mt_q[q] col range [1024 g + 128 m, +128)
                    dst = mt_q[q][:, :].rearrange(
                        "p (g i) -> p g i", g=4)[:, :, 128 * m:128 * m + 128]
                    src = trp[:, :].rearrange("p (g i) -> p g i", g=4)
                    nc.scalar.copy(out=dst, in_=src)
                if ret_pmm:
                    return pmm

            def extract_mirror(d, q):
                # mt_q[q] g-region: j's [512q+128g, +128), i in [0,1024)
                for g in range(4):
                    jg = 4 * q + g
                    for s in range(2):
                        c0 = 1024 * g + 512 * s
                        o0 = jg * 16 + s * 8
                        nc.vector.max(sv[d][:, o0:o0 + 8],
                                      mt_q[q][:, c0:c0 + 512])
                        nc.vector.max_index(si[d][:, o0:o0 + 8],
                                            sv[d][:, o0:o0 + 8],
                                            mt_q[q][:, c0:c0 + 512])

            # ---------------- merge + output per m-tile ----------------
            def do_merge(m):
                cv = cand_v[:, m * SLOTS:(m + 1) * SLOTS]
                cw = cand_w[:, m * SLOTS:(m + 1) * SLOTS]
                # pull this m-tile's remote slots (blocks 5..7) from staging
                nc.scalar.copy(out=cand_v[:, m * SLOTS + 80:m * SLOTS + 128],
                               in_=rv[:, m * 48:(m + 1) * 48])
                nc.scalar.copy(out=cand_w[:, m * SLOTS + 80:m * SLOTS + 128],
                               in_=rw[:, m * 48:(m + 1) * 48])
                ccol = p_small.tile([128, SLOTS], U16, tag="ccol")
                nc.vector.tensor_tensor(out=ccol[:, :], in0=base_iota[:, :],
                                        in1=cw[:, :], op=ALU.add)
                t24 = p_small.tile([128, 24], F32, tag="t24")
                s24 = p_small.tile([128, 24], U16, tag="s24")
                for r in range(3):
                    nc.vector.max(t24[:, 8 * r:8 * r + 8], cv[:, :])
                    nc.vector.max_index(s24[:, 8 * r:8 * r + 8],
                                        t24[:, 8 * r:8 * r + 8], cv[:, :])
                    if r < 2:
                        nc.vector.match_replace(cv[:, :],
                                                t24[:, 8 * r:8 * r + 8],
                                                cv[:, :], -1e30)
                ros = p_small.tile([128, SLOTS], I16, tag="ros")
                nc.gpsimd.local_scatter(ros[:, :], rank_iota[:, :],
                                        s24[:, :].bitcast(I16),
                                        channels=128, num_elems=SLOTS,
                                        num_idxs=24)
                nc.vector.tensor_scalar_add(ros[:, :], ros[:, :], -1)
                cols = p_small.tile([128, 24], I16, tag="cols")
                nc.gpsimd.local_scatter(cols[:, :], ccol[:, :].bitcast(I16),
                                        ros[:, :], channels=128,
                                        num_elems=24, num_idxs=SLOTS)
                nc.vector.memset(cols[:, K1:24], 32767)
                hi3 = p_small.tile([128, 24], I16, tag="hi3")
                nc.vector.tensor_scalar(hi3[:, :], cols[:, :], 10, None,
                                        op0=ALU.logical_shift_right)
                lo10 = p_small.tile([128, 24], I16, tag="lo10")
                nc.vector.tensor_scalar(lo10[:, :], cols[:, :], 1023, None,
                                        op0=ALU.bitwise_and)
                for c in range(OUT_CH):
                    idx = p_small.tile([128, 24], I16, name=f"idx{c % 2}",
                                       tag=f"idx{c % 2}")
                    nc.vector.tensor_scalar(idx[:, :], hi3[:, :], c, -2048,
                                            op0=ALU.not_equal, op1=ALU.mult)
                    nc.vector.tensor_tensor(out=idx[:, :], in0=idx[:, :],
                                            in1=lo10[:, :], op=ALU.add)
                    corr = p_corr.tile([128, 1024], F16, tag="corr")
                    nc.gpsimd.local_scatter(corr[:, :], ew16[:, :],
                                            idx[:, :], channels=128,
                                            num_elems=1024, num_idxs=24)
                    ost = p_stage.tile([128, 1024], F32, tag="ost", bufs=3)
                    nc.scalar.copy(out=ost[:, :], in_=corr[:, :])
                    nc.sync.dma_start(
                        out=out_ext[m * 128:(m + 1) * 128,
                                    c * 1024:(c + 1) * 1024],
                        in_=ost[:, :])

            # ---------------- schedule ----------------
            pid = nc.gpsimd.partition_id()

            def ship(group, deltas):
                for d in deltas:
                    dst = (pid + d) % N_CORES
                    nc.gpsimd.dma_start(
                        a2a_in[group][bass.ds(dst * 128, 128), 0:128],
                        sv[d][:, :])
                    nc.gpsimd.dma_start(
                        a2a_in[group][bass.ds(dst * 128, 128),
                                      128:192].bitcast(U16),
                        si[d][:, :])
                nc.gpsimd.collective_compute(
                    "AllToAll", ALU.bypass,
                    replica_groups=[list(range(N_CORES))],
                    ins=[a2a_in[group].ap()], outs=[a2a_out[group].ap()],
                )

            def recv(group, deltas):
                rv_r = rv[:, :].rearrange("p (m s) -> p m s", m=MT)
                rw_r = rw[:, :].rearrange("p (m s) -> p m s", m=MT)
                for d in deltas:
                    src = (pid + N_CORES - d) % N_CORES
                    b = N_CORES - d - 5          # 0..2 within staging
                    nc.gpsimd.dma_start(
                        rv_r[:, :, b * 16:(b + 1) * 16],
                        a2a_out[group][bass.ds(src * 128, 128), 0:128])
                    nc.gpsimd.dma_start(
                        rw_r[:, :, b * 16:(b + 1) * 16],
                        a2a_out[group][bass.ds(src * 128, 128),
                                       128:192].bitcast(U16))

            # interleave q0 preps of blocks 0,1 first so block1-q0
            # matmuls can start before the q1 preps finish
            for q in range(2):
                prep_rows(512 * q, xl[(0, q)], xl[(1, q)],
                          first=(q == 0), use_dve=True)
                prep_rows(R + 512 * q, xr[(0, q)], xr[(1, q)],
                          use_dve=True)
                if q == 0:
                    # diag q0 tiles for m<4 need only xl-q0: fill the
                    # prefix PE idle while block1's rhs is still prepping
                    for m in range(4):
                        do_tile(0, 0, m, mirror=False)
            NEXT_PREP = {1: 2, 2: 3, 3: 4}
            for d in (1, 2, 3):
                prep_block(NEXT_PREP[d])
                for q in range(2):
                    for m in range(MT):
                        do_tile(d, q, m, mirror=True)
                    extract_mirror(d, q)
                if d == 2:
                    ship("A", (1, 2))   # flies during block 3
                elif d == 3:
                    ship("B", (3,))     # flies during blocks 4+0
            recv("A", (1, 2))
            recv("B", (3,))
            # blocks 4 and 0 m-outer, merge interleaved: merge m fires as
            # soon as its last local extraction lands (collective already
            # in flight / landed).  Diagonal triangle: block0 (m>=4, q=0)
            # tiles are below the diagonal; their candidates come from
            # transposing the (m<4, q=1) tiles instead.
            def diag_mirror_tile(m, pmm):
                ev = p_stage.tile([128, 512], F32, tag="ev", bufs=2)
                nc.scalar.copy(out=ev[:, :], in_=pmm[:, :])
                trp = p_tr.tile([128, 512], F32, tag="trp", bufs=1)
                for g in range(4):
                    nc.tensor.transpose(trp[:, g * 128:(g + 1) * 128],
                                        ev[:, g * 128:(g + 1) * 128],
                                        ident[:, :])
                dst = mt_q[0][:, :].rearrange(
                    "p (g i) -> p g i", g=4)[:, :, 128 * m:128 * m + 128]
                src = trp[:, :].rearrange("p (g i) -> p g i", g=4)
                nc.scalar.copy(out=dst, in_=src)

            MERGE_LAG = MT
            for m in range(MT):
                for q in range(2):
                    do_tile(4, q, m, mirror=False)
                if m < 4:
                    pmm_q1 = do_tile(0, 1, m, mirror=False, ret_pmm=True)
                    diag_mirror_tile(m, pmm_q1)
                else:
                    do_tile(0, 1, m, mirror=False)
                if m == 3:
                    # fill slots for rows m'=4+g, block0 window 0
                    for g in range(4):
                        c0 = 1024 * g
                        o0 = (4 + g) * SLOTS
                        nc.vector.max(cand_v[:, o0:o0 + 8],
                                      mt_q[0][:, c0:c0 + 512])
                        nc.vector.max_index(cand_w[:, o0:o0 + 8],
                                            cand_v[:, o0:o0 + 8],
                                            mt_q[0][:, c0:c0 + 512])
                if m >= MERGE_LAG:
                    do_merge(m - MERGE_LAG)
            for m in range(MT - MERGE_LAG, MT):
                do_merge(m)

    nc.compile()
    return nc


_KERNEL_CACHE = {}


def kernel(x: np.ndarray, edge_weights: np.ndarray) -> np.ndarray:
    import os
    from concourse._compat import axon_active
    if axon_active() and os.environ.get("JAX_PLATFORMS") == "cpu":
        os.environ.pop("JAX_PLATFORMS")
    x = np.ascontiguousarray(x, dtype=np.float32)
    ew = np.ascontiguousarray(edge_weights, dtype=np.float32).reshape(1, -1)
    B, D = x.shape
    R = B // N_CORES

    key = (B, D)
    if key not in _KERNEL_CACHE:
        _KERNEL_CACHE[key] = build_kernel(B, D)
    nc = _KERNEL_CACHE[key]

    in_maps = []
    for c in range(N_CORES):
        in_maps.append({
            "xall": np.ascontiguousarray(np.roll(x, -R * c, axis=0)),
            "ew": ew,
        })
    res = run_bass_kernel_spmd(nc, in_maps, core_ids=list(range(N_CORES)))
    out = np.empty((B, B), dtype=np.float32)
    for c in range(N_CORES):
        out[c * R:(c + 1) * R, :] = np.roll(res.results[c]["out"], R * c,
                                            axis=1)
    return out
